# revision 24
# baseline (speedup 1.0000x reference)
"""Trainium2 Bass kernel for a teacher-forced attention GRU decoder.

Math (per reference):
  tokens  = [BOS, target[:, :-1]]                # [T, B]
  Ua_keys = enc @ Ua_w.T + Ua_b (+ Wa_b folded)  # [B, S, H]
  per step t:
    x      = emb[tok_t]                          # [B, H]
    q      = h @ Wa_w.T (+ Wa_b, folded into Ua_keys)
    scores = va . tanh(q[:,None,:] + Ua_keys)    # (+ Va_b: softmax-invariant, skipped)
    w      = softmax(scores, axis=s)
    ctx    = w @ enc
    gi     = [x, ctx] @ W_ih.T + b_ih ; gh = h @ W_hh.T + b_hh
    r, z   = sigmoid(gi_r + gh_r), sigmoid(gi_z + gh_z)
    n      = tanh(gi_n + r * gh_n)
    h      = n + z * (h - n)
    logits_t = h @ out_w.T + out_b
  outputs: log_softmax(logits), h_final, attention weights

Distribution: data-parallel over batch across 8 cores (16 rows each).
Each core runs the recurrence for its rows, then computes its [T*16, V]
logits block streaming out_w.T in bf16, with a local log-softmax
(no collectives needed).

Matmul dtypes: float32r (fp32 with reduced multiply precision, ~1e-4 rel)
for the attention/recurrence matmuls; bf16 for the large vocab projection
(error is absorbed by log-softmax: output magnitudes ~ -log V).
"""

import numpy as np
import ml_dtypes

B, S, H, V, T = 128, 128, 512, 32000, 5
BOS = 1
NCORES = 8
BS = B // NCORES            # batch rows per core
BSS = BS * S                # 2048 (b, s) pairs per core
HC = H // 128               # 4 h-chunks of 128
ROWS = T * BS               # 80 logit rows per core (t-major)
H2 = 2 * H                  # GRU input size (x ++ ctx)
H3 = 3 * H                  # GRU gate rows
NT = 500                    # logits N-chunk (PSUM bank limit 512 fp32)
NBLK = 2000                 # out_w streaming block (4 chunks of NT)
NCHUNKS = V // NT           # 64
P = 128

_nc_cache = {}


def _build_nc():
    if 'nc' in _nc_cache:
        return _nc_cache['nc']

    import concourse.bass as bass
    import concourse.tile as tile
    from concourse import bacc, mybir

    f32 = mybir.dt.float32
    f32r = mybir.dt.float32r
    bf16 = mybir.dt.bfloat16
    i32 = mybir.dt.int32
    AF = mybir.ActivationFunctionType
    ALU = mybir.AluOpType
    AX = mybir.AxisListType

    nc = bacc.Bacc('TRN2', target_bir_lowering=False, debug=False,
                   num_devices=NCORES)

    # ---- per-core DRAM inputs ----
    encT_d = nc.dram_tensor('encT', [H, BSS], f32, kind='ExternalInput').ap()
    enc_d = nc.dram_tensor('enc', [BSS, H], f32, kind='ExternalInput').ap()
    h0T_d = nc.dram_tensor('h0T', [H, BS], f32, kind='ExternalInput').ap()
    h0_d = nc.dram_tensor('h0', [BS, H], f32, kind='ExternalInput').ap()
    tok_d = nc.dram_tensor('tok', [ROWS, 1], i32, kind='ExternalInput').ap()
    emb_d = nc.dram_tensor('emb', [V, H], f32, kind='ExternalInput').ap()
    WaT_d = nc.dram_tensor('WaT', [H, H], f32, kind='ExternalInput').ap()
    UaT_d = nc.dram_tensor('UaT', [H, H], f32, kind='ExternalInput').ap()
    WihT_d = nc.dram_tensor('WihT', [H2, H3], f32, kind='ExternalInput').ap()
    WhhT_d = nc.dram_tensor('WhhT', [H, H3], f32, kind='ExternalInput').ap()
    va_d = nc.dram_tensor('va', [1, H], f32, kind='ExternalInput').ap()
    battn_d = nc.dram_tensor('battn', [1, H], f32, kind='ExternalInput').ap()
    bih_d = nc.dram_tensor('bih', [1, H3], f32, kind='ExternalInput').ap()
    bhh_d = nc.dram_tensor('bhh', [1, H3], f32, kind='ExternalInput').ap()
    outb_d = nc.dram_tensor('outb', [1, V], bf16, kind='ExternalInput').ap()
    owT_d = nc.dram_tensor('owT', [H, V], bf16, kind='ExternalInput').ap()

    # ---- per-core DRAM outputs (t-major; host transposes to b-major) ----
    dec_d = nc.dram_tensor('dec', [T, BS, V], f32, kind='ExternalOutput').ap()
    attn_d = nc.dram_tensor('attn', [T, BS, S], f32, kind='ExternalOutput').ap()
    hfin_d = nc.dram_tensor('hfin', [BS, H], f32, kind='ExternalOutput').ap()

    from contextlib import ExitStack
    from concourse.masks import make_identity

    with tile.TileContext(nc) as tc, ExitStack() as ctx:
        const = ctx.enter_context(tc.tile_pool(name='const', bufs=1))
        state = ctx.enter_context(tc.tile_pool(name='state', bufs=1))
        # pools for phases 0-2; closed before the logits phase frees SBUF/PSUM
        p12 = ctx.enter_context(ExitStack())
        wpool = p12.enter_context(tc.tile_pool(name='weights', bufs=1))
        psum_small = p12.enter_context(
            tc.tile_pool(name='psum_small', bufs=1, space='PSUM'))

        identity = const.tile([P, P], f32)
        make_identity(nc, identity[:])

        onesf = const.tile([1, 512], f32)
        nc.vector.memset(onesf[:], 1.0)
        ones_r = const.tile([1, 512], f32r)
        nc.vector.tensor_copy(ones_r[:], onesf[:])
        ones80_bf = const.tile([1, ROWS], bf16)
        nc.vector.memset(ones80_bf[:], 1.0)

        # resident rounded weights (GRU weights are loaded after P1 frees
        # the encT staging space)
        WaT_r = wpool.tile([P, HC * H], f32r)       # [h-part, hk*512 + h']
        enc_r = wpool.tile([P, BS * H], f32r)       # [s-part, b*512 + h]
        uakeys = wpool.tile([P, HC * BSS], f32)     # [k'-part, m*2048 + (b,s)]
        va_r = const.tile([P, HC], f32r)
        battn_row_r = const.tile([1, H], f32r)
        bih_r = const.tile([1, H3], f32r)
        bhh_r = const.tile([1, H3], f32r)

        xT_r = state.tile([P, HC * ROWS], f32r)     # [k-part, kc*80 + t*16 + b]
        hT_r = state.tile([P, HC * BS], f32r)       # [h-part, hk*16 + b]
        hTs_bf = state.tile([P, HC * ROWS], bf16)   # h states for logits lhsT
        wblk_r = state.tile([P, BS * BS], f32r)     # block-diag attn weights

        # ---------- P0: load + round weights ----------
        def load_round(dst, dst_col, src_ap, shape, pool, tag='stage'):
            st = pool.tile(list(shape), f32, tag=tag)
            nc.sync.dma_start(st[:], src_ap)
            nc.vector.tensor_copy(dst[:, dst_col:dst_col + shape[1]], st[:])

        with tc.tile_pool(name='stage', bufs=3) as stage:
            zerof = stage.tile([P, BS * BS], f32, tag='zerof')
            nc.vector.memset(zerof[:], 0.0)
            nc.vector.tensor_copy(wblk_r[:], zerof[:])
            for k in range(HC):
                load_round(WaT_r, k * H, WaT_d[k * P:(k + 1) * P, :], [P, H], stage)
            for b in range(BS):
                load_round(enc_r, b * H, enc_d[b * S:(b + 1) * S, :], [P, H], stage)

            # va as [p, hk] columns
            st = stage.tile([P, HC], f32, tag='small')
            nc.sync.dma_start(st[:], va_d[0, :].rearrange('(k p) -> p k', p=P))
            nc.vector.tensor_copy(va_r[:], st[:])
            # bias rows
            st2 = stage.tile([1, H], f32, tag='small2')
            nc.sync.dma_start(st2[:], battn_d[:, :])
            nc.vector.tensor_copy(battn_row_r[:], st2[:])
            st3 = stage.tile([1, H3], f32, tag='small3')
            nc.sync.dma_start(st3[:], bih_d[:, :])
            nc.vector.tensor_copy(bih_r[:], st3[:])
            st4 = stage.tile([1, H3], f32, tag='small4')
            nc.sync.dma_start(st4[:], bhh_d[:, :])
            nc.vector.tensor_copy(bhh_r[:], st4[:])

            # h0T -> hT_r
            for k in range(HC):
                load_round(hT_r, k * BS, h0T_d[k * P:(k + 1) * P, :], [P, BS],
                           stage, tag='small_h')

            # embedding gather for all (t, b) rows, then transpose
            tok_sb = stage.tile([ROWS, 1], i32, tag='tok')
            nc.sync.dma_start(tok_sb[:], tok_d[:, :])
            x_g = stage.tile([ROWS, H], f32, tag='gather')
            nc.gpsimd.indirect_dma_start(
                out=x_g[:], out_offset=None, in_=emb_d[:, :],
                in_offset=bass.IndirectOffsetOnAxis(ap=tok_sb[:, :1], axis=0))
            for k in range(HC):
                tp = psum_small.tile([P, ROWS], f32, tag='small')
                nc.tensor.transpose(tp[:], x_g[:, k * P:(k + 1) * P],
                                    identity[:ROWS, :ROWS])
                nc.vector.tensor_copy(xT_r[:, k * ROWS:(k + 1) * ROWS], tp[:])

        # ---------- P1: Ua_keys = enc @ Ua_w.T + (Ua_b + Wa_b) ----------
        with tc.tile_pool(name='p1stage', bufs=3) as stage, \
             tc.tile_pool(name='p1w', bufs=1) as p1w, \
             tc.tile_pool(name='p1psum', bufs=2, space='PSUM') as p1psum:
            UaT_r = p1w.tile([P, HC * H], f32r)
            encT_r = p1w.tile([P, HC * BSS], f32r)
            for k in range(HC):
                load_round(UaT_r, k * H, UaT_d[k * P:(k + 1) * P, :], [P, H], stage)
            for k in range(HC):
                load_round(encT_r, k * BSS, encT_d[k * P:(k + 1) * P, :], [P, BSS],
                           stage, tag='stage_e')
            for m in range(HC):
                for n in range(HC):
                    ps = p1psum.tile([P, 512], f32)
                    for k in range(HC):
                        nc.tensor.matmul(
                            ps[:],
                            UaT_r[:, k * H + m * P:k * H + (m + 1) * P],
                            encT_r[:, k * BSS + n * 512:k * BSS + (n + 1) * 512],
                            start=(k == 0), stop=False)
                    nc.tensor.matmul(
                        ps[:], battn_row_r[:, m * P:(m + 1) * P], ones_r[:, :512],
                        start=False, stop=True)
                    nc.any.tensor_copy(
                        uakeys[:, m * BSS + n * 512:m * BSS + (n + 1) * 512], ps[:])

        # ---------- GRU weights (after P1 staging space is free) ----------
        wg = p12.enter_context(tc.tile_pool(name='gru_w', bufs=1))
        WihT_r = wg.tile([P, 8 * H3], f32r)         # [k-part, kc*1536 + j]
        WhhT_r = wg.tile([P, HC * H3], f32r)
        with tc.tile_pool(name='wg_stage', bufs=3) as stage:
            for k in range(8):
                load_round(WihT_r, k * H3, WihT_d[k * P:(k + 1) * P, :], [P, H3],
                           stage, tag='stage_w')
            for k in range(HC):
                load_round(WhhT_r, k * H3, WhhT_d[k * P:(k + 1) * P, :], [P, H3],
                           stage, tag='stage_w')

        # ---------- P2: recurrence ----------
        rec = p12.enter_context(tc.tile_pool(name='rec', bufs=2))
        attn_sb = p12.enter_context(tc.tile_pool(name='attn_sb', bufs=2))
        psum_sc = p12.enter_context(
            tc.tile_pool(name='psum_sc', bufs=1, space='PSUM'))
        psum_g = p12.enter_context(
            tc.tile_pool(name='psum_g', bufs=1, space='PSUM'))

        h_nat = None
        for t in range(T):
            # qT[h', b] = sum_h WaT[h, h'] hT[h, b]
            qTf = rec.tile([P, HC * BS], f32, tag='qtf')
            for m in range(HC):
                psq = psum_small.tile([P, BS], f32, tag='small')
                for k in range(HC):
                    nc.tensor.matmul(
                        psq[:], WaT_r[:, k * H + m * P:k * H + (m + 1) * P],
                        hT_r[:, k * BS:(k + 1) * BS],
                        start=(k == 0), stop=(k == HC - 1))
                nc.any.tensor_copy(qTf[:, m * BS:(m + 1) * BS], psq[:])

            # scores[b, s] = va . tanh(uakeys + q)
            scs = [psum_sc.tile([1, 512], f32, tag=f'sc{n}', bufs=1,
                                name=f'scs{n}')
                   for n in range(4)]
            for m in range(HC):
                tanht = attn_sb.tile([P, BSS], f32r, tag='tanht')
                nc.vector.tensor_tensor(
                    tanht[:].rearrange('p (b s) -> p b s', s=S),
                    uakeys[:, m * BSS:(m + 1) * BSS].rearrange(
                        'p (b s) -> p b s', s=S),
                    qTf[:, m * BS:(m + 1) * BS].unsqueeze(2).broadcast_to(
                        [P, BS, S]),
                    op=ALU.add)
                nc.scalar.activation(tanht[:], tanht[:], AF.Tanh)
                for n in range(4):
                    nc.tensor.matmul(
                        scs[n][:], va_r[:, m:m + 1],
                        tanht[:, n * 512:(n + 1) * 512],
                        start=(m == 0), stop=(m == HC - 1))

            # softmax over s (scores land as [1, (b,s)] -> [16, 128])
            screv = rec.tile([1, BSS], f32, tag='screv', bufs=1)
            for n in range(4):
                if n % 2:
                    nc.scalar.activation(screv[:, n * 512:(n + 1) * 512],
                                         scs[n][:], AF.Copy)
                else:
                    nc.vector.tensor_copy(screv[:, n * 512:(n + 1) * 512],
                                          scs[n][:])
            sc = rec.tile([BS, S], f32, tag='sc_t', bufs=1)
            for n in range(4):
                # flat element order matches: src is b-major (b, s), dst
                # iterates partitions (b) then s
                nc.sync.dma_start(sc[4 * n:4 * (n + 1), :],
                                  screv[:, n * 512:(n + 1) * 512])
            negmx = rec.tile([BS, 1], f32, tag='negmx')
            nc.vector.tensor_reduce(negmx[:], sc[:], axis=AX.X, op=ALU.max,
                                    negate=True)
            ex = rec.tile([BS, S], f32, tag='ex')
            sume = rec.tile([BS, 1], f32, tag='sume')
            nc.scalar.activation(ex[:], sc[:], AF.Exp, bias=negmx[:, :1],
                                 accum_out=sume[:, :1])
            rcp = rec.tile([BS, 1], f32, tag='rcp')
            nc.vector.reciprocal(rcp[:], sume[:])
            w = rec.tile([BS, S], f32, tag='w')
            nc.vector.tensor_scalar(w[:], ex[:], rcp[:, :1], None, op0=ALU.mult)
            nc.sync.dma_start(attn_d[t, :, :], w[:])

            # wT into block-diagonal [s, b-chunk] tile (stride-17 columns)
            pswt = psum_small.tile([P, BS], f32, tag='small')
            nc.tensor.transpose(pswt[:S, :], w[:], identity[:BS, :BS])
            nc.vector.tensor_copy(wblk_r[:, 0:BS * BS:BS + 1], pswt[:S, :])

            # ctx[b, h] via block-diag lhsT chunks
            psx = psum_small.tile([BS, 512], f32, tag='ctx')
            for b in range(BS):
                nc.tensor.matmul(
                    psx[:], wblk_r[:, b * BS:(b + 1) * BS],
                    enc_r[:, b * H:(b + 1) * H],
                    start=(b == 0), stop=(b == BS - 1))
            ctxf = rec.tile([BS, H], f32, tag='ctxf', bufs=1)
            nc.any.tensor_copy(ctxf[:], psx[:])
            ctxT_r = rec.tile([P, HC * BS], f32r, tag='ctxT')
            for k in range(HC):
                pst = psum_small.tile([P, BS], f32, tag='small')
                nc.tensor.transpose(pst[:], ctxf[:, k * P:(k + 1) * P],
                                    identity[:BS, :BS])
                nc.vector.tensor_copy(ctxT_r[:, k * BS:(k + 1) * BS], pst[:])

            # gates: gi = [x, ctx] @ W_ih.T + b_ih ; gh = h @ W_hh.T + b_hh
            def gi_matmuls(ps, g, start, stop):
                for k in range(8):
                    lhsT = (xT_r[:, k * ROWS + t * BS:k * ROWS + (t + 1) * BS]
                            if k < HC else
                            ctxT_r[:, (k - HC) * BS:(k - HC + 1) * BS])
                    nc.tensor.matmul(
                        ps[:], lhsT,
                        WihT_r[:, k * H3 + g * 512:k * H3 + (g + 1) * 512],
                        start=start and (k == 0), stop=False)
                nc.tensor.matmul(ps[:], ones_r[:, :BS],
                                 bih_r[:, g * 512:(g + 1) * 512],
                                 start=False, stop=stop)

            def gh_matmuls(ps, g, start, stop):
                for k in range(HC):
                    nc.tensor.matmul(
                        ps[:], hT_r[:, k * BS:(k + 1) * BS],
                        WhhT_r[:, k * H3 + g * 512:k * H3 + (g + 1) * 512],
                        start=start and (k == 0), stop=False)
                nc.tensor.matmul(ps[:], ones_r[:, :BS],
                                 bhh_r[:, g * 512:(g + 1) * 512],
                                 start=False, stop=stop)

            # r and z: gi + gh summed directly in one PSUM bank
            ps_r = psum_g.tile([BS, H], f32, tag='gi')
            gi_matmuls(ps_r, 0, start=True, stop=False)
            gh_matmuls(ps_r, 0, start=False, stop=True)
            rg = rec.tile([BS, H], f32, tag='g_r', bufs=1)
            nc.scalar.activation(rg[:], ps_r[:], AF.Sigmoid)

            ps_z = psum_g.tile([BS, H], f32, tag='gi')
            gi_matmuls(ps_z, 1, start=True, stop=False)
            gh_matmuls(ps_z, 1, start=False, stop=True)
            zg = rec.tile([BS, H], f32, tag='g_z', bufs=1)
            nc.scalar.activation(zg[:], ps_z[:], AF.Sigmoid)

            # n needs r * gh_n, so gh stays separate
            gi_n = psum_g.tile([BS, H], f32, tag='gi')
            gi_matmuls(gi_n, 2, start=True, stop=True)
            gh_n = psum_g.tile([BS, H], f32, tag='gh')
            gh_matmuls(gh_n, 2, start=True, stop=True)
            ng = rec.tile([BS, H], f32, tag='g_n', bufs=1)
            nc.vector.tensor_tensor(ng[:], gh_n[:], rg[:], op=ALU.mult)
            nc.vector.tensor_tensor(ng[:], gi_n[:], ng[:], op=ALU.add)
            nc.scalar.activation(ng[:], ng[:], AF.Tanh)

            # h_new = n + z * (h - n)
            if t == 0:
                h_nat = rec.tile([BS, H], f32, tag='hnew')
                nc.sync.dma_start(h_nat[:], h0_d[:, :])
            hmn = rec.tile([BS, H], f32, tag='hmn', bufs=1)
            nc.vector.tensor_tensor(hmn[:], h_nat[:], ng[:], op=ALU.subtract)
            nc.vector.tensor_tensor(hmn[:], zg[:], hmn[:], op=ALU.mult)
            h_new = rec.tile([BS, H], f32, tag='hnew')
            nc.vector.tensor_tensor(h_new[:], ng[:], hmn[:], op=ALU.add)
            h_nat = h_new

            # transpose h_new -> hT_r (next step) and hTs_bf (logits lhsT)
            for k in range(HC):
                pst = psum_small.tile([P, BS], f32, tag='small')
                nc.tensor.transpose(pst[:], h_new[:, k * P:(k + 1) * P],
                                    identity[:BS, :BS])
                nc.vector.tensor_copy(hT_r[:, k * BS:(k + 1) * BS], pst[:])
                nc.vector.tensor_copy(
                    hTs_bf[:, k * ROWS + t * BS:k * ROWS + (t + 1) * BS], pst[:])
            if t == T - 1:
                nc.sync.dma_start(hfin_d[:, :], h_new[:])

        # ---------- P3: logits + log_softmax ----------
        p12.close()  # free phase-0/1/2 SBUF + PSUM for the logits phase
        with tc.tile_pool(name='lpool', bufs=1) as lpool, \
             tc.tile_pool(name='stream', bufs=8) as stream, \
             tc.tile_pool(name='lwork', bufs=3) as lwork, \
             tc.tile_pool(name='psum_l', bufs=2, space='PSUM') as psum_l:
            logits_bf = lpool.tile([ROWS, V], bf16)
            sums = lpool.tile([ROWS, NCHUNKS], f32)
            for blk in range(V // NBLK):
                c0 = blk * NBLK
                owt = []
                for k in range(HC):
                    ow = stream.tile([P, NBLK], bf16, tag='ow')
                    nc.sync.dma_start(
                        ow[:], owT_d[k * P:(k + 1) * P, c0:c0 + NBLK])
                    owt.append(ow)
                obs = stream.tile([1, NBLK], bf16, tag='outbs', bufs=2)
                nc.sync.dma_start(obs[:], outb_d[:, c0:c0 + NBLK])
                for sub in range(NBLK // NT):
                    c = c0 + sub * NT
                    chunk = blk * (NBLK // NT) + sub
                    psL = psum_l.tile([ROWS, NT], f32)
                    for k in range(HC):
                        nc.tensor.matmul(
                            psL[:], hTs_bf[:, k * ROWS:(k + 1) * ROWS],
                            owt[k][:, sub * NT:(sub + 1) * NT],
                            start=(k == 0), stop=False)
                    nc.tensor.matmul(psL[:], ones80_bf[:],
                                     obs[:, sub * NT:(sub + 1) * NT],
                                     start=False, stop=True)
                    scr = lwork.tile([ROWS, NT], f32, tag='escr')
                    nc.scalar.activation(scr[:], psL[:], AF.Exp,
                                         accum_out=sums[:, chunk:chunk + 1])
                    nc.vector.tensor_copy(logits_bf[:, c:c + NT], psL[:])
            ssum = lpool.tile([ROWS, 1], f32)
            nc.vector.tensor_reduce(ssum[:], sums[:], axis=AX.X, op=ALU.add)
            lse = lpool.tile([ROWS, 1], f32)
            nc.scalar.activation(lse[:], ssum[:], AF.Ln)
            neglse = lpool.tile([ROWS, 1], f32)
            nc.vector.tensor_scalar(neglse[:], lse[:], -1.0, None, op0=ALU.mult)
            dec_flat = dec_d.rearrange('t b v -> (t b) v')
            for chunk in range(NCHUNKS):
                c = chunk * NT
                fin = lwork.tile([ROWS, NT], f32, tag='fin')
                nc.vector.tensor_scalar(fin[:], logits_bf[:, c:c + NT],
                                        neglse[:, :1], None, op0=ALU.add)
                nc.sync.dma_start(dec_flat[:, c:c + NT], fin[:])

    nc.compile()
    _nc_cache['nc'] = nc
    return nc


def _prep_in_maps(encoder_outputs, encoder_hidden, target_tensor, emb, Wa_w,
                  Wa_b, Ua_w, Ua_b, Va_w, Va_b, W_ih, b_ih, W_hh, b_hh,
                  out_w, out_b):
    f32 = np.float32
    bf16 = ml_dtypes.bfloat16
    enc = np.ascontiguousarray(np.asarray(encoder_outputs, dtype=f32))
    h0 = np.asarray(encoder_hidden, dtype=f32)[0]              # [B, H]
    tgt = np.asarray(target_tensor).astype(np.int32)           # [B, T]
    tokens = np.concatenate(
        [np.full((1, B), BOS, np.int32), tgt.T[:-1]], axis=0)  # [T, B]

    WaT = np.ascontiguousarray(np.asarray(Wa_w, f32).T)
    UaT = np.ascontiguousarray(np.asarray(Ua_w, f32).T)
    WihT = np.ascontiguousarray(np.asarray(W_ih, f32).T)
    WhhT = np.ascontiguousarray(np.asarray(W_hh, f32).T)
    owT_bf = np.ascontiguousarray(np.asarray(out_w, f32).T).astype(bf16)
    battn = (np.asarray(Wa_b, f32) + np.asarray(Ua_b, f32))[None]  # [1, H]
    va = np.asarray(Va_w, f32).reshape(1, H)
    emb_f = np.ascontiguousarray(np.asarray(emb, f32))
    outb_bf = np.asarray(out_b, f32).reshape(1, V).astype(bf16)
    bih = np.asarray(b_ih, f32).reshape(1, H3)
    bhh = np.asarray(b_hh, f32).reshape(1, H3)

    in_maps = []
    for c in range(NCORES):
        b0 = c * BS
        enc_sh = enc[b0:b0 + BS]                               # [BS, S, H]
        in_maps.append({
            'encT': np.ascontiguousarray(
                enc_sh.transpose(2, 0, 1).reshape(H, BSS)),
            'enc': np.ascontiguousarray(enc_sh.reshape(BSS, H)),
            'h0T': np.ascontiguousarray(h0[b0:b0 + BS].T),
            'h0': np.ascontiguousarray(h0[b0:b0 + BS]),
            'tok': np.ascontiguousarray(
                tokens[:, b0:b0 + BS].reshape(ROWS, 1)),
            'emb': emb_f,
            'WaT': WaT, 'UaT': UaT, 'WihT': WihT, 'WhhT': WhhT,
            'va': va, 'battn': battn, 'bih': bih, 'bhh': bhh,
            'outb': outb_bf, 'owT': owT_bf,
        })
    return in_maps


def kernel(**inputs):
    from concourse.bass_utils import run_bass_kernel_spmd
    nc = _build_nc()
    in_maps = _prep_in_maps(**inputs)
    res = run_bass_kernel_spmd(nc, in_maps, core_ids=list(range(NCORES)))
    decs, attns, hfins = [], [], []
    for r in res.results:
        decs.append(r['dec'].transpose(1, 0, 2))     # [BS, T, V]
        attns.append(r['attn'].transpose(1, 0, 2))   # [BS, T, S]
        hfins.append(r['hfin'])
    decoder_outputs = np.concatenate(decs, axis=0)
    attentions = np.concatenate(attns, axis=0)
    h_final = np.concatenate(hfins, axis=0)[None]
    return decoder_outputs, h_final, attentions


# revision 32
# speedup vs baseline: 1.0626x; 1.0626x over previous
"""Trainium2 Bass kernel for a teacher-forced attention GRU decoder.

Math (per reference):
  tokens  = [BOS, target[:, :-1]]                # [T, B]
  Ua_keys = enc @ Ua_w.T + Ua_b (+ Wa_b folded)  # [B, S, H]
  per step t:
    x      = emb[tok_t]                          # [B, H]
    q      = h @ Wa_w.T (+ Wa_b, folded into Ua_keys)
    scores = va . tanh(q[:,None,:] + Ua_keys)    # (+ Va_b: softmax-invariant, skipped)
    w      = softmax(scores, axis=s)
    ctx    = w @ enc
    gi     = [x, ctx] @ W_ih.T + b_ih ; gh = h @ W_hh.T + b_hh
    r, z   = sigmoid(gi_r + gh_r), sigmoid(gi_z + gh_z)
    n      = tanh(gi_n + r * gh_n)
    h      = n + z * (h - n)
    logits_t = h @ out_w.T + out_b
  outputs: log_softmax(logits), h_final, attention weights

Distribution: data-parallel over batch across 8 cores (16 rows each).
Each core runs the recurrence for its rows, then computes its [T*16, V]
logits block streaming out_w.T in bf16, with a local log-softmax
(no collectives needed).

Matmul dtypes: float32r (fp32 with reduced multiply precision, ~1e-4 rel)
for the attention/recurrence matmuls; bf16 for the large vocab projection
(error is absorbed by log-softmax: output magnitudes ~ -log V).
"""

import numpy as np
import ml_dtypes

B, S, H, V, T = 128, 128, 512, 32000, 5
BOS = 1
NCORES = 8
BS = B // NCORES            # batch rows per core
BSS = BS * S                # 2048 (b, s) pairs per core
HC = H // 128               # 4 h-chunks of 128
ROWS = T * BS               # 80 logit rows per core (t-major)
H2 = 2 * H                  # GRU input size (x ++ ctx)
H3 = 3 * H                  # GRU gate rows
NT = 500                    # logits N-chunk (PSUM bank limit 512 fp32)
NBLK = 4000                 # out_w streaming block (8 chunks of NT)
NFIN = 2000                 # finalize/output chunk
NCHUNKS = V // NT           # 64
P = 128

_nc_cache = {}


def _build_nc():
    if 'nc' in _nc_cache:
        return _nc_cache['nc']

    import concourse.bass as bass
    import concourse.tile as tile
    from concourse import bacc, mybir

    f32 = mybir.dt.float32
    f32r = mybir.dt.float32r
    bf16 = mybir.dt.bfloat16
    i32 = mybir.dt.int32
    AF = mybir.ActivationFunctionType
    ALU = mybir.AluOpType
    AX = mybir.AxisListType

    nc = bacc.Bacc('TRN2', target_bir_lowering=False, debug=False,
                   num_devices=NCORES)

    # ---- per-core DRAM inputs ----
    encT_d = nc.dram_tensor('encT', [H, BSS], f32, kind='ExternalInput').ap()
    enc_d = nc.dram_tensor('enc', [BSS, H], f32, kind='ExternalInput').ap()
    h0T_d = nc.dram_tensor('h0T', [H, BS], f32, kind='ExternalInput').ap()
    h0_d = nc.dram_tensor('h0', [BS, H], f32, kind='ExternalInput').ap()
    tok_d = nc.dram_tensor('tok', [ROWS, 1], i32, kind='ExternalInput').ap()
    emb_d = nc.dram_tensor('emb', [V, H], f32, kind='ExternalInput').ap()
    WaT_d = nc.dram_tensor('WaT', [H, H], f32, kind='ExternalInput').ap()
    UaT_d = nc.dram_tensor('UaT', [H, H], f32, kind='ExternalInput').ap()
    WihT_d = nc.dram_tensor('WihT', [H2, H3], f32, kind='ExternalInput').ap()
    WhhT_d = nc.dram_tensor('WhhT', [H, H3], f32, kind='ExternalInput').ap()
    va_d = nc.dram_tensor('va', [1, H], f32, kind='ExternalInput').ap()
    battn_d = nc.dram_tensor('battn', [1, H], f32, kind='ExternalInput').ap()
    bih_d = nc.dram_tensor('bih', [1, H3], f32, kind='ExternalInput').ap()
    bhh_d = nc.dram_tensor('bhh', [1, H3], f32, kind='ExternalInput').ap()
    outb_d = nc.dram_tensor('outb', [1, V], bf16, kind='ExternalInput').ap()
    owT_d = nc.dram_tensor('owT', [H, V], bf16, kind='ExternalInput').ap()

    # ---- per-core DRAM outputs (t-major; host transposes to b-major) ----
    dec_d = nc.dram_tensor('dec', [T, BS, V], f32, kind='ExternalOutput').ap()
    attn_d = nc.dram_tensor('attn', [T, BS, S], f32, kind='ExternalOutput').ap()
    hfin_d = nc.dram_tensor('hfin', [BS, H], f32, kind='ExternalOutput').ap()

    from contextlib import ExitStack
    from concourse.masks import make_identity

    with tile.TileContext(nc) as tc, ExitStack() as ctx:
        const = ctx.enter_context(tc.tile_pool(name='const', bufs=1))
        state = ctx.enter_context(tc.tile_pool(name='state', bufs=1))
        dramp = ctx.enter_context(tc.tile_pool(name='dramp', bufs=1,
                                               space='DRAM'))
        # pools for phases 0-2; closed before the logits phase frees SBUF/PSUM
        p12 = ctx.enter_context(ExitStack())
        wpool = p12.enter_context(tc.tile_pool(name='weights', bufs=1))
        psum_small = p12.enter_context(
            tc.tile_pool(name='psum_small', bufs=1, space='PSUM'))

        identity = const.tile([P, P], f32)
        make_identity(nc, identity[:])

        onesf = const.tile([1, 512], f32)
        nc.vector.memset(onesf[:], 1.0)
        ones_r = const.tile([1, 512], f32r)
        nc.vector.tensor_copy(ones_r[:], onesf[:])
        ones80_bf = const.tile([1, ROWS], bf16)
        nc.vector.memset(ones80_bf[:], 1.0)

        # resident rounded weights (GRU weights are loaded after P1 frees
        # the encT staging space)
        WaT_r = wpool.tile([P, HC * H], f32r)       # [h-part, hk*512 + h']
        enc_r = wpool.tile([P, BS * H], f32r)       # [s-part, b*512 + h]
        uakeys = wpool.tile([P, HC * BSS], f32)     # [k'-part, m*2048 + (b,s)]
        va_r = const.tile([P, HC], f32r)
        battn_row_r = const.tile([1, H], f32r)
        bhhn_r = const.tile([1, 512], f32r)         # b_hh n-gate slice

        xT_r = state.tile([P, HC * ROWS], f32r)     # [k-part, kc*80 + t*16 + b]
        hT_r = state.tile([P, HC * BS], f32r)       # [h-part, hk*16 + b]
        hTs_bf = state.tile([P, HC * ROWS], bf16)   # h states for logits lhsT
        wblk_r = state.tile([P, BS * BS], f32r)     # block-diag attn weights
        gix_d = dramp.tile([ROWS, H3], f32)         # x @ W_ih_x.T + biases

        # ---------- P0: load + round weights (merged big DMAs) ----------
        def load_round(dst, src_ap, shape, pool, tag):
            st = pool.tile(list(shape), f32, tag=tag)
            nc.sync.dma_start(st[:], src_ap)
            nc.vector.tensor_copy(dst, st[:])

        with tc.tile_pool(name='stage', bufs=1) as stage:
            zerof = stage.tile([P, BS * BS], f32, tag='zerof', bufs=1)
            nc.vector.memset(zerof[:], 0.0)
            nc.vector.tensor_copy(wblk_r[:], zerof[:])

            load_round(WaT_r[:], WaT_d.rearrange('(k p) h -> p k h', p=P),
                       [P, HC * H], stage, 'stage_w')
            load_round(enc_r[:], enc_d.rearrange('(b s) h -> s b h', s=S),
                       [P, BS * H], stage, 'stage_e')
            load_round(hT_r[:], h0T_d.rearrange('(k p) b -> p k b', p=P),
                       [P, HC * BS], stage, 'small_h')

            # va as [p, hk] columns
            st = stage.tile([P, HC], f32, tag='small')
            nc.sync.dma_start(st[:], va_d.rearrange('o (k p) -> p (o k)', p=P))
            nc.vector.tensor_copy(va_r[:], st[:])
            # bias rows
            st2 = stage.tile([1, H], f32, tag='small2')
            nc.sync.dma_start(st2[:], battn_d[:, :])
            nc.vector.tensor_copy(battn_row_r[:], st2[:])

            # embedding gather for all (t, b) rows, then transpose
            tok_sb = stage.tile([ROWS, 1], i32, tag='tok')
            nc.sync.dma_start(tok_sb[:], tok_d[:, :])
            x_g = stage.tile([ROWS, H], f32, tag='gather', bufs=1)
            nc.gpsimd.indirect_dma_start(
                out=x_g[:], out_offset=None, in_=emb_d[:, :],
                in_offset=bass.IndirectOffsetOnAxis(ap=tok_sb[:, :1], axis=0))
            for k in range(HC):
                tp = psum_small.tile([P, ROWS], f32, tag='small')
                nc.tensor.transpose(tp[:], x_g[:, k * P:(k + 1) * P],
                                    identity[:ROWS, :ROWS])
                nc.vector.tensor_copy(xT_r[:, k * ROWS:(k + 1) * ROWS], tp[:])

        # ---------- P1: Ua_keys = enc @ Ua_w.T + (Ua_b + Wa_b) ----------
        with tc.tile_pool(name='p1stage', bufs=1) as stage, \
             tc.tile_pool(name='p1w', bufs=1) as p1w, \
             tc.tile_pool(name='p1psum', bufs=2, space='PSUM') as p1psum:
            UaT_r = p1w.tile([P, HC * H], f32r)
            encT_r = p1w.tile([P, HC * BSS], f32r)
            load_round(UaT_r[:], UaT_d.rearrange('(k p) h -> p k h', p=P),
                       [P, HC * H], stage, 'stage_u')
            load_round(encT_r[:], encT_d.rearrange('(k p) q -> p k q', p=P),
                       [P, HC * BSS], stage, 'stage_et')
            for m in range(HC):
                for n in range(HC):
                    ps = p1psum.tile([P, 512], f32)
                    for k in range(HC):
                        nc.tensor.matmul(
                            ps[:],
                            UaT_r[:, k * H + m * P:k * H + (m + 1) * P],
                            encT_r[:, k * BSS + n * 512:k * BSS + (n + 1) * 512],
                            start=(k == 0), stop=False)
                    nc.tensor.matmul(
                        ps[:], battn_row_r[:, m * P:(m + 1) * P], ones_r[:, :512],
                        start=False, stop=True)
                    nc.any.tensor_copy(
                        uakeys[:, m * BSS + n * 512:m * BSS + (n + 1) * 512], ps[:])

        # ---------- GRU weights (after P1 staging space is free) ----------
        wg = p12.enter_context(tc.tile_pool(name='gru_w', bufs=1))
        WihT_r = wg.tile([P, 8 * H3], f32r)         # [k-part, kc*1536 + j]
        WhhT_r = wg.tile([P, HC * H3], f32r)
        with tc.tile_pool(name='wg_stage', bufs=1) as stage:
            load_round(WihT_r[:, :4 * H3],
                       WihT_d[:4 * P].rearrange('(k p) j -> p k j', p=P),
                       [P, 4 * H3], stage, 'stage_w')
            load_round(WihT_r[:, 4 * H3:],
                       WihT_d[4 * P:].rearrange('(k p) j -> p k j', p=P),
                       [P, 4 * H3], stage, 'stage_w')
            load_round(WhhT_r[:],
                       WhhT_d.rearrange('(k p) j -> p k j', p=P),
                       [P, HC * H3], stage, 'stage_w')

        # gi_x[(t,b), j] = x @ W_ih_x.T + b_ih (+ b_hh for r/z gates);
        # step-invariant, spilled to DRAM and re-read per step.
        with tc.tile_pool(name='gix_stage', bufs=1) as stage, \
             tc.tile_pool(name='gix_psum', bufs=2, space='PSUM') as gix_psum:
            bihs = stage.tile([1, H3], f32, tag='biha')
            nc.sync.dma_start(bihs[:], bih_d[:, :])
            bhhs = stage.tile([1, H3], f32, tag='bihb')
            nc.sync.dma_start(bhhs[:], bhh_d[:, :])
            bsums = stage.tile([1, H3], f32, tag='bihc')
            nc.vector.tensor_tensor(bsums[:], bihs[:], bhhs[:], op=ALU.add)
            bih_sr = stage.tile([1, H3], f32r, tag='bihd')
            nc.vector.tensor_copy(bih_sr[:], bihs[:])
            bsum_sr = stage.tile([1, H3], f32r, tag='bihe')
            nc.vector.tensor_copy(bsum_sr[:], bsums[:])
            nc.vector.tensor_copy(bhhn_r[:], bhhs[:, 2 * 512:3 * 512])
            gixs = stage.tile([ROWS, H3], f32, tag='gixs')
            for g in range(3):
                ps = gix_psum.tile([ROWS, 512], f32, tag='gix')
                for k in range(HC):
                    nc.tensor.matmul(
                        ps[:], xT_r[:, k * ROWS:(k + 1) * ROWS],
                        WihT_r[:, k * H3 + g * 512:k * H3 + (g + 1) * 512],
                        start=(k == 0), stop=False)
                # + b_ih (+ b_hh for r/z: gh sums into the same PSUM later)
                brow = bsum_sr if g < 2 else bih_sr
                nc.tensor.matmul(ps[:], ones_r[:, :ROWS],
                                 brow[:, g * 512:(g + 1) * 512],
                                 start=False, stop=True)
                nc.any.tensor_copy(gixs[:, g * 512:(g + 1) * 512], ps[:])
            nc.sync.dma_start(gix_d[:, :], gixs[:])

        # ---------- P2: recurrence ----------
        rec = p12.enter_context(tc.tile_pool(name='rec', bufs=2))
        attn_sb = p12.enter_context(tc.tile_pool(name='attn_sb', bufs=2))
        psum_sc = p12.enter_context(
            tc.tile_pool(name='psum_sc', bufs=1, space='PSUM'))
        psum_g = p12.enter_context(
            tc.tile_pool(name='psum_g', bufs=1, space='PSUM'))

        h_nat = None
        for t in range(T):
            # gi_x rows for this step
            gxt = rec.tile([BS, H3], f32, tag='gxt', bufs=1)
            nc.sync.dma_start(gxt[:], gix_d[t * BS:(t + 1) * BS, :])

            # qT[h', b] = sum_h WaT[h, h'] hT[h, b]
            qTf = rec.tile([P, HC * BS], f32, tag='qtf')
            for m in range(HC):
                psq = psum_small.tile([P, BS], f32, tag='small')
                for k in range(HC):
                    nc.tensor.matmul(
                        psq[:], WaT_r[:, k * H + m * P:k * H + (m + 1) * P],
                        hT_r[:, k * BS:(k + 1) * BS],
                        start=(k == 0), stop=(k == HC - 1))
                nc.any.tensor_copy(qTf[:, m * BS:(m + 1) * BS], psq[:])

            # scores[b, s] = va . tanh(uakeys + q)
            scs = [psum_sc.tile([1, 512], f32, tag=f'sc{n}', bufs=1,
                                name=f'scs{n}')
                   for n in range(4)]
            for m in range(HC):
                tanht = attn_sb.tile([P, BSS], f32r, tag='tanht')
                nc.vector.tensor_tensor(
                    tanht[:].rearrange('p (b s) -> p b s', s=S),
                    uakeys[:, m * BSS:(m + 1) * BSS].rearrange(
                        'p (b s) -> p b s', s=S),
                    qTf[:, m * BS:(m + 1) * BS].unsqueeze(2).broadcast_to(
                        [P, BS, S]),
                    op=ALU.add)
                nc.scalar.activation(tanht[:], tanht[:], AF.Tanh)
                for n in range(4):
                    nc.tensor.matmul(
                        scs[n][:], va_r[:, m:m + 1],
                        tanht[:, n * 512:(n + 1) * 512],
                        start=(m == 0), stop=(m == HC - 1))

            # softmax over s (scores land as [1, (b,s)] -> [16, 128])
            screv = rec.tile([1, BSS], f32, tag='screv', bufs=1)
            for n in range(4):
                if n % 2:
                    nc.scalar.activation(screv[:, n * 512:(n + 1) * 512],
                                         scs[n][:], AF.Copy)
                else:
                    nc.vector.tensor_copy(screv[:, n * 512:(n + 1) * 512],
                                          scs[n][:])
            sc = rec.tile([BS, S], f32, tag='sc_t', bufs=1)
            # flat element order matches: src is b-major (b, s), dst iterates
            # partitions (b) then s
            nc.sync.dma_start(sc[:], screv[:])
            negmx = rec.tile([BS, 1], f32, tag='negmx')
            nc.vector.tensor_reduce(negmx[:], sc[:], axis=AX.X, op=ALU.max,
                                    negate=True)
            ex = rec.tile([BS, S], f32, tag='ex')
            sume = rec.tile([BS, 1], f32, tag='sume')
            nc.scalar.activation(ex[:], sc[:], AF.Exp, bias=negmx[:, :1],
                                 accum_out=sume[:, :1])
            rcp = rec.tile([BS, 1], f32, tag='rcp')
            nc.vector.reciprocal(rcp[:], sume[:])
            w = rec.tile([BS, S], f32, tag='w')
            nc.vector.tensor_scalar(w[:], ex[:], rcp[:, :1], None, op0=ALU.mult)
            nc.sync.dma_start(attn_d[t, :, :], w[:])

            # wT into block-diagonal [s, b-chunk] tile (stride-17 columns)
            pswt = psum_small.tile([P, BS], f32, tag='small')
            nc.tensor.transpose(pswt[:S, :], w[:], identity[:BS, :BS])
            nc.vector.tensor_copy(wblk_r[:, 0:BS * BS:BS + 1], pswt[:S, :])

            # ctx[b, h] via block-diag lhsT chunks
            psx = psum_small.tile([BS, 512], f32, tag='ctx')
            for b in range(BS):
                nc.tensor.matmul(
                    psx[:], wblk_r[:, b * BS:(b + 1) * BS],
                    enc_r[:, b * H:(b + 1) * H],
                    start=(b == 0), stop=(b == BS - 1))
            ctxf = rec.tile([BS, H], f32, tag='ctxf', bufs=1)
            nc.any.tensor_copy(ctxf[:], psx[:])
            ctxT_r = rec.tile([P, HC * BS], f32r, tag='ctxT')
            for k in range(HC):
                pst = psum_small.tile([P, BS], f32, tag='small')
                nc.tensor.transpose(pst[:], ctxf[:, k * P:(k + 1) * P],
                                    identity[:BS, :BS])
                nc.vector.tensor_copy(ctxT_r[:, k * BS:(k + 1) * BS], pst[:])

            # gates; gi_ctx and gh accumulate into one PSUM for r/z
            # (all biases except b_hh_n live in the precomputed gi_x)
            def gictx_matmuls(ps, g, start, stop):
                for k in range(HC):
                    nc.tensor.matmul(
                        ps[:], ctxT_r[:, k * BS:(k + 1) * BS],
                        WihT_r[:, (k + HC) * H3 + g * 512:
                               (k + HC) * H3 + (g + 1) * 512],
                        start=start and (k == 0),
                        stop=stop and (k == HC - 1))

            def gh_matmuls(ps, g, start, stop):
                for k in range(HC):
                    nc.tensor.matmul(
                        ps[:], hT_r[:, k * BS:(k + 1) * BS],
                        WhhT_r[:, k * H3 + g * 512:k * H3 + (g + 1) * 512],
                        start=start and (k == 0),
                        stop=stop and (k == HC - 1))

            # r and z: (gi_ctx + gh) in one PSUM bank, then + gi_x, sigmoid
            ps_r = psum_g.tile([BS, H], f32, tag='gi')
            gictx_matmuls(ps_r, 0, start=True, stop=False)
            gh_matmuls(ps_r, 0, start=False, stop=True)
            rg = rec.tile([BS, H], f32, tag='g_r', bufs=1)
            nc.vector.tensor_tensor(rg[:], ps_r[:], gxt[:, :512], op=ALU.add)
            nc.scalar.activation(rg[:], rg[:], AF.Sigmoid)

            ps_z = psum_g.tile([BS, H], f32, tag='gi')
            gictx_matmuls(ps_z, 1, start=True, stop=False)
            gh_matmuls(ps_z, 1, start=False, stop=True)
            zg = rec.tile([BS, H], f32, tag='g_z', bufs=1)
            nc.vector.tensor_tensor(zg[:], ps_z[:], gxt[:, 512:1024],
                                    op=ALU.add)
            nc.scalar.activation(zg[:], zg[:], AF.Sigmoid)

            # n = tanh(gi_x_n + gi_ctx_n + r * (gh_n + b_hh_n))
            gi_n = psum_g.tile([BS, H], f32, tag='gi')
            gictx_matmuls(gi_n, 2, start=True, stop=True)  # includes b_hh_n?
            gh_n = psum_g.tile([BS, H], f32, tag='gh')
            gh_matmuls(gh_n, 2, start=True, stop=False)
            nc.tensor.matmul(gh_n[:], ones_r[:, :BS], bhhn_r[:, :512],
                             start=False, stop=True)
            ng = rec.tile([BS, H], f32, tag='g_n', bufs=1)
            nc.vector.tensor_tensor(ng[:], gh_n[:], rg[:], op=ALU.mult)
            nc.vector.tensor_tensor(ng[:], gi_n[:], ng[:], op=ALU.add)
            nc.vector.tensor_tensor(ng[:], ng[:], gxt[:, 1024:1536],
                                    op=ALU.add)
            nc.scalar.activation(ng[:], ng[:], AF.Tanh)

            # h_new = n + z * (h - n)
            if t == 0:
                h_nat = rec.tile([BS, H], f32, tag='hnew')
                nc.sync.dma_start(h_nat[:], h0_d[:, :])
            hmn = rec.tile([BS, H], f32, tag='hmn', bufs=1)
            nc.vector.tensor_tensor(hmn[:], h_nat[:], ng[:], op=ALU.subtract)
            nc.vector.tensor_tensor(hmn[:], zg[:], hmn[:], op=ALU.mult)
            h_new = rec.tile([BS, H], f32, tag='hnew')
            nc.vector.tensor_tensor(h_new[:], ng[:], hmn[:], op=ALU.add)
            h_nat = h_new

            # transpose h_new -> hT_r (next step) and hTs_bf (logits lhsT)
            for k in range(HC):
                pst = psum_small.tile([P, BS], f32, tag='small')
                nc.tensor.transpose(pst[:], h_new[:, k * P:(k + 1) * P],
                                    identity[:BS, :BS])
                nc.vector.tensor_copy(hT_r[:, k * BS:(k + 1) * BS], pst[:])
                nc.vector.tensor_copy(
                    hTs_bf[:, k * ROWS + t * BS:k * ROWS + (t + 1) * BS], pst[:])
            if t == T - 1:
                nc.sync.dma_start(hfin_d[:, :], h_new[:])

        # ---------- P3: logits + log_softmax ----------
        p12.close()  # free phase-0/1/2 SBUF + PSUM for the logits phase
        with tc.tile_pool(name='lpool', bufs=1) as lpool, \
             tc.tile_pool(name='stream', bufs=2) as stream, \
             tc.tile_pool(name='lwork', bufs=3) as lwork, \
             tc.tile_pool(name='psum_l', bufs=2, space='PSUM') as psum_l:
            logits_bf = lpool.tile([ROWS, V], bf16)
            sums = lpool.tile([ROWS, NCHUNKS], f32)
            for blk in range(V // NBLK):
                c0 = blk * NBLK
                owt = []
                for k in range(HC):
                    ow = stream.tile([P, NBLK], bf16, tag=f'ow{k}',
                                     name=f'ow{k}')
                    half = NBLK // 2
                    nc.sync.dma_start(
                        ow[:, :half], owT_d[k * P:(k + 1) * P, c0:c0 + half])
                    nc.sync.dma_start(
                        ow[:, half:],
                        owT_d[k * P:(k + 1) * P, c0 + half:c0 + NBLK])
                    owt.append(ow)
                obs = stream.tile([1, NBLK], bf16, tag='outbs')
                nc.sync.dma_start(obs[:], outb_d[:, c0:c0 + NBLK])
                for sub in range(NBLK // NT):
                    c = c0 + sub * NT
                    chunk = blk * (NBLK // NT) + sub
                    psL = psum_l.tile([ROWS, NT], f32)
                    for k in range(HC):
                        nc.tensor.matmul(
                            psL[:], hTs_bf[:, k * ROWS:(k + 1) * ROWS],
                            owt[k][:, sub * NT:(sub + 1) * NT],
                            start=(k == 0), stop=False)
                    nc.tensor.matmul(psL[:], ones80_bf[:],
                                     obs[:, sub * NT:(sub + 1) * NT],
                                     start=False, stop=True)
                    scr = lwork.tile([ROWS, NT], f32, tag='escr')
                    nc.scalar.activation(scr[:], psL[:], AF.Exp,
                                         accum_out=sums[:, chunk:chunk + 1])
                    nc.vector.tensor_copy(logits_bf[:, c:c + NT], psL[:])
            ssum = lpool.tile([ROWS, 1], f32)
            nc.vector.tensor_reduce(ssum[:], sums[:], axis=AX.X, op=ALU.add)
            lse = lpool.tile([ROWS, 1], f32)
            nc.scalar.activation(lse[:], ssum[:], AF.Ln)
            neglse = lpool.tile([ROWS, 1], f32)
            nc.vector.tensor_scalar(neglse[:], lse[:], -1.0, None, op0=ALU.mult)
            dec_flat = dec_d.rearrange('t b v -> (t b) v')
            for fi in range(V // NFIN):
                c = fi * NFIN
                fin = lwork.tile([ROWS, NFIN], f32, tag='fin', bufs=2)
                if fi % 2:
                    nc.scalar.activation(fin[:], logits_bf[:, c:c + NFIN],
                                         AF.Identity, bias=neglse[:, :1])
                else:
                    nc.vector.tensor_scalar(fin[:], logits_bf[:, c:c + NFIN],
                                            neglse[:, :1], None, op0=ALU.add)
                nc.sync.dma_start(dec_flat[:, c:c + NFIN], fin[:])

    nc.compile()
    _nc_cache['nc'] = nc
    return nc


def _prep_in_maps(encoder_outputs, encoder_hidden, target_tensor, emb, Wa_w,
                  Wa_b, Ua_w, Ua_b, Va_w, Va_b, W_ih, b_ih, W_hh, b_hh,
                  out_w, out_b):
    f32 = np.float32
    bf16 = ml_dtypes.bfloat16
    enc = np.ascontiguousarray(np.asarray(encoder_outputs, dtype=f32))
    h0 = np.asarray(encoder_hidden, dtype=f32)[0]              # [B, H]
    tgt = np.asarray(target_tensor).astype(np.int32)           # [B, T]
    tokens = np.concatenate(
        [np.full((1, B), BOS, np.int32), tgt.T[:-1]], axis=0)  # [T, B]

    WaT = np.ascontiguousarray(np.asarray(Wa_w, f32).T)
    UaT = np.ascontiguousarray(np.asarray(Ua_w, f32).T)
    WihT = np.ascontiguousarray(np.asarray(W_ih, f32).T)
    WhhT = np.ascontiguousarray(np.asarray(W_hh, f32).T)
    owT_bf = np.ascontiguousarray(np.asarray(out_w, f32).T).astype(bf16)
    battn = (np.asarray(Wa_b, f32) + np.asarray(Ua_b, f32))[None]  # [1, H]
    va = np.asarray(Va_w, f32).reshape(1, H)
    emb_f = np.ascontiguousarray(np.asarray(emb, f32))
    outb_bf = np.asarray(out_b, f32).reshape(1, V).astype(bf16)
    bih = np.asarray(b_ih, f32).reshape(1, H3)
    bhh = np.asarray(b_hh, f32).reshape(1, H3)

    in_maps = []
    for c in range(NCORES):
        b0 = c * BS
        enc_sh = enc[b0:b0 + BS]                               # [BS, S, H]
        in_maps.append({
            'encT': np.ascontiguousarray(
                enc_sh.transpose(2, 0, 1).reshape(H, BSS)),
            'enc': np.ascontiguousarray(enc_sh.reshape(BSS, H)),
            'h0T': np.ascontiguousarray(h0[b0:b0 + BS].T),
            'h0': np.ascontiguousarray(h0[b0:b0 + BS]),
            'tok': np.ascontiguousarray(
                tokens[:, b0:b0 + BS].reshape(ROWS, 1)),
            'emb': emb_f,
            'WaT': WaT, 'UaT': UaT, 'WihT': WihT, 'WhhT': WhhT,
            'va': va, 'battn': battn, 'bih': bih, 'bhh': bhh,
            'outb': outb_bf, 'owT': owT_bf,
        })
    return in_maps


def kernel(**inputs):
    from concourse.bass_utils import run_bass_kernel_spmd
    nc = _build_nc()
    in_maps = _prep_in_maps(**inputs)
    res = run_bass_kernel_spmd(nc, in_maps, core_ids=list(range(NCORES)))
    decs, attns, hfins = [], [], []
    for r in res.results:
        decs.append(r['dec'].transpose(1, 0, 2))     # [BS, T, V]
        attns.append(r['attn'].transpose(1, 0, 2))   # [BS, T, S]
        hfins.append(r['hfin'])
    decoder_outputs = np.concatenate(decs, axis=0)
    attentions = np.concatenate(attns, axis=0)
    h_final = np.concatenate(hfins, axis=0)[None]
    return decoder_outputs, h_final, attentions
